# revision 8
# baseline (speedup 1.0000x reference)
"""CP-decomposed 3x3 conv on 8 TRN2 NeuronCores.

Math: out[f,i,j] = sum_{h,w,c,r} in[c,i+h,j+w] * f1[h,r] * f2[w,r] * f3[c,r] * f0[f,r]

Factorization used on-device (per core, over its 32-row slice of output):
  stage A: t2[r, n]  = sum_h sum_c (f3[c,r]*f1[h,r]) * x[c, n + h*W]     (3 matmuls, K=C)
  stage B: out[f, n] = sum_w sum_r (f2[w,r]*f0[f,r]) * t2[r, n + w]      (3 matmuls, K=R)
where n flattens (row, col) with row pitch W=256; output cols 254/255 of each
row are garbage (the host slices them off).

Per-core layout (all fp16 on device, fp32 PSUM accumulation):
  - SBUF partition half 0 holds input rows [0,18) of the core's 34-row
    window, half 1 holds rows [16,34) -- a block split, NOT a duplicate, so
    the input is read from HBM once.
  - Each quad iteration computes 4 chunks (a chunk = 512 output positions =
    2 output rows): A,B = chunks 2q,2q+1 (from half 0) and C,D = chunks
    8+2q,8+2q+1 (from half 1).
  - Stage A runs the 4 chunks on 4 disjoint 64x64 PE quadrants concurrently
    (A=(0,0), B=(0,64), C=(64,0), D=(64,64) as (row,col) tile positions,
    auto-derived from AP base partitions).  t2 partition half 0 <- {A,C},
    half 1 <- {B,D}.  Chunks need not be adjacent in t2: the w-shift reads
    that cross a chunk boundary only feed output cols 254/255, which are
    dropped.
  - Stage B runs 2-wide (row-split k=0/1 over t2 partition halves), two
    serial bank slots g, filling the whole PE array.
  - PSUM evacuation is split between vector and scalar engines; outputs are
    cast to fp16 in the copy and DMAed (full 256-col rows for >=512B
    descriptor runs).

Sharding: output rows (Ho=254) split across 8 cores: cores 0-6 get rows
[32i, 32i+32); core 7 processes rows [222, 254) via a shifted window (its
first 2 rows duplicate core 6's tail and are dropped at gather).
"""

import sys

sys.path.insert(0, "/opt/trn_rl_repo")

import numpy as np

# Problem constants (hardcoded per contract)
C = 64
H = 256
W = 256
FH = 3
FW = 3
RANK = 64
F = 128
HO = H - FH + 1  # 254
WO = W - FW + 1  # 254
NCORES = 8
ROWS = 32  # output rows per core
IN_ROWS = ROWS + 2
HALF_OUT_ROWS = ROWS // 2  # 16
HALF_IN_ROWS = HALF_OUT_ROWS + 2  # 18
XCOLS = HALF_IN_ROWS * W  # columns of each X2 partition half
CHUNK = 512  # output positions per matmul (= 2 rows x 256), one PSUM bank
NCHUNK = ROWS // 2  # 16
NQUAD = NCHUNK // 4  # 4

# Compute dtype for matmul operands: "fp16" | "bf16" | "fp32"
COMPUTE_DT = "fp16"
# Output DRAM dtype: "fp16" | "fp32"
OUT_DT = "fp16"
# Ablation switches for benchmarking: subset of
# {"in_dma", "out_dma", "stage_a", "stage_b", "copies", "all"}
ABLATE = set()

_PROGRAM_CACHE = {}


def _np_dt(name):
    if name == "fp16":
        return np.dtype(np.float16)
    if name == "bf16":
        import ml_dtypes

        return np.dtype(ml_dtypes.bfloat16)
    return np.dtype(np.float32)


def build_program(
    compute_dt=None,
    out_dt=None,
    num_devices=NCORES,
    reps=1,
    bench_internal=False,
):
    """Build + compile the per-core Bass program.

    reps>1 wraps the body in a device-side hardware loop (benchmarking only).
    bench_internal puts the real I/O on internal DRAM scratch so the host
    transfer per call is tiny (timing runs only).
    """
    from concourse import bacc, mybir, tile

    compute_dt = compute_dt or COMPUTE_DT
    out_dt = out_dt or OUT_DT
    dt_map = {
        "fp16": mybir.dt.float16,
        "bf16": mybir.dt.bfloat16,
        "fp32": mybir.dt.float32,
    }
    dt_c = dt_map[compute_dt]
    dt_o = dt_map[out_dt]
    dt_f32 = mybir.dt.float32

    nc = bacc.Bacc(
        "TRN2", target_bir_lowering=False, debug=False, num_devices=num_devices
    )
    if bench_internal:
        x = nc.dram_tensor("x_int", [C, IN_ROWS, W], dt_c).ap()
        wa = nc.dram_tensor("wa_int", [C, FH * RANK], dt_c).ap()
        wb = nc.dram_tensor("wb_int", [RANK, FW * F], dt_c).ap()
        y = nc.dram_tensor("y_int", [F, ROWS, W], dt_o).ap()
        tin = nc.dram_tensor("tin", [1, 16], dt_f32, kind="ExternalInput").ap()
        tout = nc.dram_tensor("tout", [1, 16], dt_f32, kind="ExternalOutput").ap()
    else:
        x = nc.dram_tensor("x", [C, IN_ROWS, W], dt_c, kind="ExternalInput").ap()
        wa = nc.dram_tensor("wa", [C, FH * RANK], dt_c, kind="ExternalInput").ap()
        wb = nc.dram_tensor("wb", [RANK, FW * F], dt_c, kind="ExternalInput").ap()
        y = nc.dram_tensor("y", [F, ROWS, W], dt_o, kind="ExternalOutput").ap()

    with tile.TileContext(nc) as tc:
        with (
            tc.tile_pool(name="xin", bufs=1) as xin_pool,
            tc.tile_pool(name="wgt", bufs=1) as wgt_pool,
            tc.tile_pool(name="t2", bufs=3) as t2_pool,
            tc.tile_pool(name="ot", bufs=4) as ot_pool,
            tc.tile_pool(name="p1", bufs=4, space="PSUM") as p1_pool,
            tc.tile_pool(name="p2", bufs=2, space="PSUM") as p2_pool,
        ):
            if bench_internal:
                nc.sync.dma_start(out=tout[:], in_=tin[:])

            def body():
                X2 = xin_pool.tile([2 * C, XCOLS], dt_c)
                WA2 = wgt_pool.tile([2 * C, FH * RANK], dt_c, tag="wa")
                WB2 = wgt_pool.tile([2 * RANK, FW * F], dt_c, tag="wb")
                # Weights on the SWDGE (gpsimd) ring so the input DMAs lead
                # the sync HWDGE ring.
                for half in range(2):
                    nc.gpsimd.dma_start(
                        out=WA2[half * C : (half + 1) * C, :], in_=wa[:, :]
                    )
                    nc.gpsimd.dma_start(
                        out=WB2[half * RANK : (half + 1) * RANK, :], in_=wb[:, :]
                    )
                if "in_dma" in ABLATE:
                    nc.vector.memset(X2[:, 0:8], 0.0)
                else:
                    xf = x.rearrange("c h w -> c (h w)")
                    # Quad-0-critical rows of BOTH halves first, then tails.
                    for a, b in ((0, 7), (7, HALF_IN_ROWS)):
                        for half in range(2):
                            r0 = HALF_OUT_ROWS * half
                            nc.sync.dma_start(
                                out=X2[half * C : (half + 1) * C, a * W : b * W],
                                in_=xf[:, (r0 + a) * W : (r0 + b) * W],
                            )

                def stage_a(q):
                    # 4 chunks on 4 disjoint PE quadrants per tap.
                    # (rhs half, psum col half, bank slot):
                    #   A=(0,0,0) B=(0,1,0) C=(1,0,1) D=(1,1,1)
                    # Two single-bank PSUM tiles so the vector and scalar
                    # evacuation copies run in parallel.
                    p1a = p1_pool.tile([2 * C, CHUNK], dt_f32, tag="p1")
                    p1b = p1_pool.tile([2 * C, CHUNK], dt_f32, tag="p1")
                    p1 = (p1a, p1b)
                    if "stage_a" in ABLATE:
                        nc.vector.memset(p1a[:, 0:8], 0.0)
                        nc.vector.memset(p1b[:, 0:8], 0.0)
                    else:
                        for t in range(FH):
                            for rh, ch, g in (
                                (0, 0, 0),
                                (0, 1, 0),
                                (1, 0, 1),
                                (1, 1, 1),
                            ):
                                # chunk local index j = 2q + ch; col offset
                                # (2j + t) * W
                                col = (4 * q + 2 * ch + t) * W
                                nc.tensor.matmul(
                                    out=p1[g][ch * C : ch * C + C, :],
                                    lhsT=WA2[
                                        rh * C : (rh + 1) * C,
                                        t * RANK : (t + 1) * RANK,
                                    ],
                                    rhs=X2[
                                        rh * C : (rh + 1) * C, col : col + CHUNK
                                    ],
                                    start=(t == 0),
                                    stop=(t == FH - 1),
                                    skip_group_check=True,
                                )
                    t2q = t2_pool.tile([2 * RANK, 2 * CHUNK + 4], dt_c, tag="t2")
                    nc.gpsimd.memset(t2q[:, 2 * CHUNK : 2 * CHUNK + 4], 0.0)
                    if "copies" in ABLATE:
                        nc.vector.memset(t2q[:, 0:8], 0.0)
                    else:
                        nc.vector.tensor_copy(out=t2q[:, 0:CHUNK], in_=p1a[:])
                        nc.scalar.copy(
                            out=t2q[:, CHUNK : 2 * CHUNK], in_=p1b[:]
                        )
                    return t2q

                def stage_b(q, t2q):
                    p2k0 = p2_pool.tile([F, 2 * CHUNK], dt_f32, tag="p2")
                    p2k1 = p2_pool.tile([F, 2 * CHUNK], dt_f32, tag="p2")
                    p2k = (p2k0, p2k1)
                    if "stage_b" in ABLATE:
                        nc.vector.memset(p2k0[:, 0:8], 0.0)
                        nc.vector.memset(p2k1[:, 0:8], 0.0)
                    else:
                        for w in range(FW):
                            for k, g in ((0, 0), (1, 0), (0, 1), (1, 1)):
                                nc.tensor.matmul(
                                    out=p2k[k][:, g * CHUNK : (g + 1) * CHUNK],
                                    lhsT=WB2[
                                        k * RANK : (k + 1) * RANK,
                                        w * F : (w + 1) * F,
                                    ],
                                    rhs=t2q[
                                        k * RANK : (k + 1) * RANK,
                                        g * CHUNK + w : g * CHUNK + w + CHUNK,
                                    ],
                                    start=(w == 0),
                                    stop=(w == FW - 1),
                                    skip_group_check=True,
                                )
                    if "out_dma" in ABLATE:
                        return
                    ots = []
                    for k in range(2):
                        ot = ot_pool.tile([F, 2 * CHUNK], dt_o)
                        if "copies" in ABLATE:
                            nc.vector.memset(ot[:, 0:8], 0.0)
                        elif k == 0:
                            nc.vector.tensor_copy(out=ot[:], in_=p2k[k][:])
                        else:
                            nc.scalar.copy(out=ot[:], in_=p2k[k][:])
                        ots.append(ot)
                    # k tile holds chunks (2q+k | 8+2q+k): global output rows
                    # {4q+2k, 4q+2k+1} and {16+4q+2k, 16+4q+2k+1}.
                    yv = y.rearrange("f (b r) w -> f b r w", b=2)
                    for k in range(2):
                        ov = ots[k].rearrange("f (b r w) -> f b r w", b=2, w=W)
                        r0 = 4 * q + 2 * k
                        nc.sync.dma_start(
                            out=yv[:, :, r0 : r0 + 2, :], in_=ov[:]
                        )

                if "all" in ABLATE:
                    junk = t2_pool.tile([RANK, CHUNK], dt_c, tag="t2")
                    nc.vector.memset(junk[:, 0:8], 0.0)
                    return
                pending = None
                for q in range(NQUAD + 1):
                    t2q = stage_a(q) if q < NQUAD else None
                    if pending is not None:
                        stage_b(q - 1, pending)
                    pending = t2q

            if reps == 1:
                body()
            else:
                # Benchmarking loop. The body exceeds one IRAM block on PE
                # (96 matmuls + sems), so hint the back-edge prefetch;
                # staggered_reset avoids the ~2us all-engine barrier.
                with tc.For_i(
                    0,
                    reps,
                    1,
                    hint_engines=(mybir.EngineType.PE,),
                    staggered_reset=True,
                ):
                    body()

    nc.compile()
    return nc


def _get_program():
    key = (COMPUTE_DT, OUT_DT)
    if key not in _PROGRAM_CACHE:
        _PROGRAM_CACHE[key] = build_program()
    return _PROGRAM_CACHE[key]


def make_weight_inputs(factor0, factor1, factor2, factor3, np_dt=None):
    np_dt = np_dt or _np_dt(COMPUTE_DT)
    f0 = np.asarray(factor0, np.float32)
    f1 = np.asarray(factor1, np.float32)
    f2 = np.asarray(factor2, np.float32)
    f3 = np.asarray(factor3, np.float32)
    # wa[c, t*R + r] = f3[c,r] * f1[t,r]
    wa = (
        (f1[:, None, :] * f3[None, :, :]).transpose(1, 0, 2).reshape(C, FH * RANK)
    ).astype(np_dt)
    # wb[r, w*F + f] = f2[w,r] * f0[f,r]
    wb = (
        (f2[:, :, None] * f0.T[None, :, :]).transpose(1, 0, 2).reshape(RANK, FW * F)
    ).astype(np_dt)
    return np.ascontiguousarray(wa), np.ascontiguousarray(wb)


ROW_STARTS = [0, 32, 64, 96, 128, 160, 192, 222]


def make_in_maps(input, factor0, factor1, factor2, factor3):
    wa, wb = make_weight_inputs(factor0, factor1, factor2, factor3)
    np_dt = _np_dt(COMPUTE_DT)
    x16 = np.asarray(input).astype(np_dt)
    return [
        {
            "x": np.ascontiguousarray(x16[:, s : s + IN_ROWS, :]),
            "wa": wa,
            "wb": wb,
        }
        for s in ROW_STARTS
    ]


def kernel(input, factor0, factor1, factor2, factor3):
    from concourse.bass_utils import run_bass_kernel_spmd

    nc = _get_program()
    in_maps = make_in_maps(input, factor0, factor1, factor2, factor3)
    res = run_bass_kernel_spmd(nc, in_maps, list(range(NCORES))).results
    out = np.empty((F, HO, WO), np.float32)
    for i, s in enumerate(ROW_STARTS):
        ys = np.asarray(res[i]["y"])[:, :, 0:WO].astype(np.float32)
        if i < NCORES - 1:
            out[:, s : s + ROWS, :] = ys
        else:
            out[:, 224:HO, :] = ys[:, 2:ROWS, :]
    return out
